# revision 8
# baseline (speedup 1.0000x reference)
"""Dilated tanh-RNN stack (5 layers, dil 1,2,4,8,16) on 8 trn2 cores.

Sharding: data-parallel over batch B=256 -> 32 per core. Time recurrence
is local. Layout on device: feature-major [H=128 partitions, T*BL cols],
col = tau*BL + b  (tau = original time). With this layout the dilation
reshape [T,B,C]->[T/d, d*B, C] is the identity on columns, so all five
layers operate in place on one SBUF buffer.

Per layer: pre = Wih@x computed by batched 512-col matmuls into a PSUM
bank (start=True); recurrence matmul Whh@h_{t-1} accumulates into the
bank slice (start=False); ScalarE Tanh(psum + b) writes h_t back to the
activation buffer (in place).
"""

import numpy as np

T, B, H, EMB, OUT = 1024, 256, 128, 10, 8
DIL = (1, 2, 4, 8, 16)
NCORES = 8
BL = B // NCORES           # 32 batch per core
COLS = T * BL              # 32768 columns
NSTRIP = 4                 # x0 packed as 4 strips of 32 partitions
STRIP_COLS = COLS // NSTRIP  # 8192
BANK = 512                 # fp32 cols per PSUM bank
NCHUNK = COLS // BANK      # 64 chunks per layer
PROJ_COLS = 10 * BL        # last 10 timesteps

_cache = {}


def _build():
    import concourse.mybir as mybir
    import concourse.tile as tile
    from concourse import bacc

    f32 = mybir.dt.float32
    AF = mybir.ActivationFunctionType

    from contextlib import ExitStack

    nc = bacc.Bacc(None, target_bir_lowering=False, debug=False)
    with tile.TileContext(nc) as tc, ExitStack() as es:
        if True:
            dram = es.enter_context(tc.tile_pool(name="dram", bufs=1, space="DRAM"))
            x0_d = dram.tile([128, STRIP_COLS], f32, kind="ExternalInput", uniquify=False, name="x0")
            w0_d = dram.tile([128, H], f32, kind="ExternalInput", uniquify=False, name="w0T")
            wih_d = dram.tile([128, 4 * H], f32, kind="ExternalInput", uniquify=False, name="wihT")
            whh_d = dram.tile([128, 5 * H], f32, kind="ExternalInput", uniquify=False, name="whhT")
            bs_d = dram.tile([128, 5], f32, kind="ExternalInput", uniquify=False, name="bsum")
            wp_d = dram.tile([128, OUT], f32, kind="ExternalInput", uniquify=False, name="wpT")
            bp_d = dram.tile([OUT, 1], f32, kind="ExternalInput", uniquify=False, name="bp")
            y_d = dram.tile([OUT, PROJ_COLS], f32, kind="ExternalOutput", uniquify=False, name="y")

            cpool = es.enter_context(tc.tile_pool(name="const", bufs=1))
            x0 = cpool.tile([128, STRIP_COLS], f32, name="x0sb")
            w0 = cpool.tile([128, H], f32, name="w0sb")
            wih = cpool.tile([128, 4 * H], f32, name="wihsb")
            whh = cpool.tile([128, 5 * H], f32, name="whhsb")
            bs = cpool.tile([128, 5], f32, name="bssb")
            wp = cpool.tile([128, OUT], f32, name="wpsb")
            bp = cpool.tile([OUT, 1], f32, name="bpsb")
            A = cpool.tile([128, COLS], f32, name="acts")
            ysb = cpool.tile([OUT, PROJ_COLS], f32, name="ysb")

            # weight/bias loads first, then x0 strip loads (layer 0 consumes
            # strip s only after strip s-1's chunks, so split the DMA)
            nc.sync.dma_start(w0[:], w0_d[:])
            nc.sync.dma_start(wih[:], wih_d[:])
            nc.sync.dma_start(whh[:], whh_d[:])
            nc.sync.dma_start(bs[:], bs_d[:])
            nc.sync.dma_start(wp[:], wp_d[:])
            nc.sync.dma_start(bp[:], bp_d[:])
            for s in range(NSTRIP):
                q = STRIP_COLS // NSTRIP
                for ss in range(NSTRIP):
                    nc.sync.dma_start(
                        x0[32 * s : 32 * s + EMB, ss * q : (ss + 1) * q],
                        x0_d[32 * s : 32 * s + EMB, ss * q : (ss + 1) * q],
                    )

            pools = []
            for l, nb in enumerate((2, 2, 2, 1, 1)):
                pools.append(
                    es.enter_context(
                        tc.tile_pool(name=f"ps{l}", bufs=nb, space="PSUM")
                    )
                )

            for l in range(5):
                d = DIL[l]
                R = d * BL                # cols per step
                steps = T // d
                spc = BANK // R           # steps per chunk (>=1)
                whh_l = whh[:, l * H : (l + 1) * H]
                bias_l = bs[:, l : l + 1]
                for c in range(NCHUNK):
                    pt = pools[l].tile([128, BANK], f32, name=f"psum{l}", tag=f"pt{l}")
                    lo = c * BANK
                    t0 = c * spc
                    nrec = spc - 1 if t0 == 0 else spc
                    # pre-activation matmul(s) for this bank
                    if l == 0:
                        s = lo // STRIP_COLS
                        off = lo % STRIP_COLS
                        nc.tensor.matmul(
                            pt[:],
                            w0[32 * s : 32 * s + EMB, :],
                            x0[32 * s : 32 * s + EMB, off : off + BANK],
                            start=True,
                            stop=(nrec == 0),
                            tile_position=(32 * s, 0),
                        )
                    else:
                        nc.tensor.matmul(
                            pt[:],
                            wih[:, (l - 1) * H : l * H],
                            A[:, lo : lo + BANK],
                            start=True,
                            stop=(nrec == 0),
                        )
                    for k in range(spc):
                        t = t0 + k
                        sl = pt[:, k * R : (k + 1) * R]
                        if t > 0:
                            nc.tensor.matmul(
                                sl,
                                whh_l,
                                A[:, (t - 1) * R : t * R],
                                start=False,
                                stop=(k == spc - 1),
                            )
                        nc.scalar.activation(
                            A[:, t * R : (t + 1) * R], sl, AF.Tanh, bias=bias_l
                        )

            # projection: y = Wp @ acts[:, -10 steps] + bp
            pp = pools[0].tile([OUT, BANK], f32, name="psproj", tag="pt0")
            nc.tensor.matmul(
                pp[:, :PROJ_COLS],
                wp[:],
                A[:, COLS - PROJ_COLS :],
                start=True,
                stop=True,
            )
            nc.scalar.activation(ysb[:], pp[:, :PROJ_COLS], AF.Identity, bias=bp[:])
            nc.sync.dma_start(y_d[:], ysb[:])

    nc.compile()
    return nc


def _get_nc():
    if "nc" not in _cache:
        _cache["nc"] = _build()
    return _cache["nc"]


def _prep_inputs(input, embed, Wih0, Wih, Whh, bih, bhh, Wp, bp):
    input = np.asarray(input)
    embed = np.asarray(embed, np.float32)
    b = (np.asarray(bih, np.float32) + np.asarray(bhh, np.float32))  # [5, H]

    w0T = np.zeros((128, H), np.float32)
    for s in range(NSTRIP):
        w0T[32 * s : 32 * s + EMB, :] = np.asarray(Wih0, np.float32).T
    wihT = np.concatenate(
        [np.asarray(Wih[i], np.float32).T for i in range(4)], axis=1
    )  # [128, 4H]
    whhT = np.concatenate(
        [np.asarray(Whh[i], np.float32).T for i in range(5)], axis=1
    )  # [128, 5H]
    bsum = np.ascontiguousarray(b.T)  # [H, 5] -> [128, 5]
    wpT = np.ascontiguousarray(np.asarray(Wp, np.float32).T)  # [128, 8]
    bpc = np.asarray(bp, np.float32).reshape(OUT, 1)

    shared = dict(
        w0T=w0T, wihT=np.ascontiguousarray(wihT), whhT=np.ascontiguousarray(whhT),
        bsum=bsum, wpT=wpT, bp=bpc,
    )

    in_maps = []
    for core in range(NCORES):
        tok = input[:, core * BL : (core + 1) * BL]          # [T, BL]
        xe = embed[tok]                                      # [T, BL, EMB]
        xe = xe.transpose(2, 0, 1).reshape(EMB, COLS)        # col = tau*BL + b
        x0 = np.zeros((128, STRIP_COLS), np.float32)
        for s in range(NSTRIP):
            x0[32 * s : 32 * s + EMB, :] = xe[:, s * STRIP_COLS : (s + 1) * STRIP_COLS]
        in_maps.append(dict(shared, x0=x0))
    return in_maps


def kernel(input, embed, Wih0, Wih, Whh, bih, bhh, Wp, bp):
    from concourse.bass_utils import run_bass_kernel_spmd

    nc = _get_nc()
    in_maps = _prep_inputs(input, embed, Wih0, Wih, Whh, bih, bhh, Wp, bp)
    res = run_bass_kernel_spmd(nc, in_maps, core_ids=list(range(NCORES)))
    _cache["last_res"] = res
    out = np.empty((10, B, OUT), np.float32)
    for core in range(NCORES):
        y = res.results[core]["y"]                 # [8, 10*BL]
        out[:, core * BL : (core + 1) * BL, :] = (
            y.reshape(OUT, 10, BL).transpose(1, 2, 0)
        )
    return out


# revision 10
# speedup vs baseline: 1.5385x; 1.5385x over previous
"""Dilated tanh-RNN stack (5 layers, dil 1,2,4,8,16) on 8 trn2 cores.

Sharding: data-parallel over batch B=256 -> 32 per core. Time recurrence
is local. Layout on device: feature-major [H=128 partitions, T*BL cols],
col = tau*BL + b  (tau = original time). With this layout the dilation
reshape [T,B,C]->[T/d, d*B, C] is the identity on columns, so all five
layers operate in place on one SBUF buffer.

Per layer: pre = Wih@x computed by batched 512-col matmuls into a PSUM
bank (start=True); recurrence matmul Whh@h_{t-1} accumulates into the
bank slice (start=False); ScalarE Tanh(psum + b) writes h_t back to the
activation buffer (in place).
"""

import numpy as np

T, B, H, EMB, OUT = 1024, 256, 128, 10, 8
DIL = (1, 2, 4, 8, 16)
NCORES = 8
BL = B // NCORES           # 32 batch per core
COLS = T * BL              # 32768 columns
NSTRIP = 4                 # x0 packed as 4 strips of 32 partitions
STRIP_COLS = COLS // NSTRIP  # 8192
BANK = 512                 # fp32 cols per PSUM bank
NCHUNK = COLS // BANK      # 64 chunks per layer
PROJ_COLS = 10 * BL        # last 10 timesteps

_cache = {}


def _build():
    import concourse.mybir as mybir
    import concourse.tile as tile
    from concourse import bacc

    f32 = mybir.dt.float32
    MMDT = mybir.dt.float32r
    AF = mybir.ActivationFunctionType

    from contextlib import ExitStack

    nc = bacc.Bacc(None, target_bir_lowering=False, debug=False)
    with tile.TileContext(nc) as tc, ExitStack() as es:
        if True:
            dram = es.enter_context(tc.tile_pool(name="dram", bufs=1, space="DRAM"))
            x0_d = dram.tile([128, STRIP_COLS], MMDT, kind="ExternalInput", uniquify=False, name="x0")
            w0_d = dram.tile([128, H], MMDT, kind="ExternalInput", uniquify=False, name="w0T")
            wih_d = dram.tile([128, 4 * H], MMDT, kind="ExternalInput", uniquify=False, name="wihT")
            whh_d = dram.tile([128, 5 * H], MMDT, kind="ExternalInput", uniquify=False, name="whhT")
            bs_d = dram.tile([128, 5], f32, kind="ExternalInput", uniquify=False, name="bsum")
            wp_d = dram.tile([128, OUT], MMDT, kind="ExternalInput", uniquify=False, name="wpT")
            bp_d = dram.tile([OUT, 1], f32, kind="ExternalInput", uniquify=False, name="bp")
            y_d = dram.tile([OUT, PROJ_COLS], f32, kind="ExternalOutput", uniquify=False, name="y")

            cpool = es.enter_context(tc.tile_pool(name="const", bufs=1))
            x0 = cpool.tile([128, STRIP_COLS], MMDT, name="x0sb")
            w0 = cpool.tile([128, H], MMDT, name="w0sb")
            wih = cpool.tile([128, 4 * H], MMDT, name="wihsb")
            whh = cpool.tile([128, 5 * H], MMDT, name="whhsb")
            bs = cpool.tile([128, 5], f32, name="bssb")
            wp = cpool.tile([128, OUT], MMDT, name="wpsb")
            bp = cpool.tile([OUT, 1], f32, name="bpsb")
            A = cpool.tile([128, COLS], MMDT, name="acts")
            ysb = cpool.tile([OUT, PROJ_COLS], f32, name="ysb")

            # weight/bias loads first, then x0 strip loads (layer 0 consumes
            # strip s only after strip s-1's chunks, so split the DMA)
            nc.sync.dma_start(w0[:], w0_d[:])
            nc.sync.dma_start(wih[:], wih_d[:])
            nc.sync.dma_start(whh[:], whh_d[:])
            nc.sync.dma_start(bs[:], bs_d[:])
            nc.sync.dma_start(wp[:], wp_d[:])
            nc.sync.dma_start(bp[:], bp_d[:])
            for s in range(NSTRIP):
                q = STRIP_COLS // NSTRIP
                for ss in range(NSTRIP):
                    nc.sync.dma_start(
                        x0[32 * s : 32 * s + EMB, ss * q : (ss + 1) * q],
                        x0_d[32 * s : 32 * s + EMB, ss * q : (ss + 1) * q],
                    )

            pools = []
            for l, nb in enumerate((2, 2, 2, 1, 1)):
                pools.append(
                    es.enter_context(
                        tc.tile_pool(name=f"ps{l}", bufs=nb, space="PSUM")
                    )
                )

            for l in range(5):
                d = DIL[l]
                R = d * BL                # cols per step
                steps = T // d
                spc = BANK // R           # steps per chunk (>=1)
                whh_l = whh[:, l * H : (l + 1) * H]
                bias_l = bs[:, l : l + 1]
                for c in range(NCHUNK):
                    pt = pools[l].tile([128, BANK], f32, name=f"psum{l}", tag=f"pt{l}")
                    lo = c * BANK
                    t0 = c * spc
                    nrec = spc - 1 if t0 == 0 else spc
                    # pre-activation matmul(s) for this bank
                    if l == 0:
                        s = lo // STRIP_COLS
                        off = lo % STRIP_COLS
                        nc.tensor.matmul(
                            pt[:],
                            w0[32 * s : 32 * s + EMB, :],
                            x0[32 * s : 32 * s + EMB, off : off + BANK],
                            start=True,
                            stop=(nrec == 0),
                            tile_position=(32 * s, 0),
                        )
                    else:
                        nc.tensor.matmul(
                            pt[:],
                            wih[:, (l - 1) * H : l * H],
                            A[:, lo : lo + BANK],
                            start=True,
                            stop=(nrec == 0),
                        )
                    for k in range(spc):
                        t = t0 + k
                        sl = pt[:, k * R : (k + 1) * R]
                        if t > 0:
                            nc.tensor.matmul(
                                sl,
                                whh_l,
                                A[:, (t - 1) * R : t * R],
                                start=False,
                                stop=(k == spc - 1),
                            )
                        nc.scalar.activation(
                            A[:, t * R : (t + 1) * R], sl, AF.Tanh, bias=bias_l
                        )

            # projection: y = Wp @ acts[:, -10 steps] + bp
            pp = pools[0].tile([OUT, BANK], f32, name="psproj", tag="pt0")
            nc.tensor.matmul(
                pp[:, :PROJ_COLS],
                wp[:],
                A[:, COLS - PROJ_COLS :],
                start=True,
                stop=True,
            )
            nc.scalar.activation(ysb[:], pp[:, :PROJ_COLS], AF.Identity, bias=bp[:])
            nc.sync.dma_start(y_d[:], ysb[:])

    nc.compile()
    return nc


def _get_nc():
    if "nc" not in _cache:
        _cache["nc"] = _build()
    return _cache["nc"]


def _prep_inputs(input, embed, Wih0, Wih, Whh, bih, bhh, Wp, bp):
    input = np.asarray(input)
    embed = np.asarray(embed, np.float32)
    b = (np.asarray(bih, np.float32) + np.asarray(bhh, np.float32))  # [5, H]

    w0T = np.zeros((128, H), np.float32)
    for s in range(NSTRIP):
        w0T[32 * s : 32 * s + EMB, :] = np.asarray(Wih0, np.float32).T
    wihT = np.concatenate(
        [np.asarray(Wih[i], np.float32).T for i in range(4)], axis=1
    )  # [128, 4H]
    whhT = np.concatenate(
        [np.asarray(Whh[i], np.float32).T for i in range(5)], axis=1
    )  # [128, 5H]
    bsum = np.ascontiguousarray(b.T)  # [H, 5] -> [128, 5]
    wpT = np.ascontiguousarray(np.asarray(Wp, np.float32).T)  # [128, 8]
    bpc = np.asarray(bp, np.float32).reshape(OUT, 1)

    shared = dict(
        w0T=w0T, wihT=np.ascontiguousarray(wihT), whhT=np.ascontiguousarray(whhT),
        bsum=bsum, wpT=wpT, bp=bpc,
    )

    in_maps = []
    for core in range(NCORES):
        tok = input[:, core * BL : (core + 1) * BL]          # [T, BL]
        xe = embed[tok]                                      # [T, BL, EMB]
        xe = xe.transpose(2, 0, 1).reshape(EMB, COLS)        # col = tau*BL + b
        x0 = np.zeros((128, STRIP_COLS), np.float32)
        for s in range(NSTRIP):
            x0[32 * s : 32 * s + EMB, :] = xe[:, s * STRIP_COLS : (s + 1) * STRIP_COLS]
        in_maps.append(dict(shared, x0=x0))
    return in_maps


def kernel(input, embed, Wih0, Wih, Whh, bih, bhh, Wp, bp):
    from concourse.bass_utils import run_bass_kernel_spmd

    nc = _get_nc()
    in_maps = _prep_inputs(input, embed, Wih0, Wih, Whh, bih, bhh, Wp, bp)
    res = run_bass_kernel_spmd(nc, in_maps, core_ids=list(range(NCORES)))
    _cache["last_res"] = res
    out = np.empty((10, B, OUT), np.float32)
    for core in range(NCORES):
        y = res.results[core]["y"]                 # [8, 10*BL]
        out[:, core * BL : (core + 1) * BL, :] = (
            y.reshape(OUT, 10, BL).transpose(1, 2, 0)
        )
    return out


# revision 11
# speedup vs baseline: 1.9677x; 1.2790x over previous
"""Dilated tanh-RNN stack (5 layers, dil 1,2,4,8,16) on 8 trn2 cores.

Sharding: data-parallel over batch B=256 -> 32 per core. Time recurrence
is local. Layout on device: feature-major [H=128 partitions, T*BL cols],
col = tau*BL + b  (tau = original time). With this layout the dilation
reshape [T,B,C]->[T/d, d*B, C] is the identity on columns, so all five
layers operate in place on one SBUF buffer.

Per layer: pre = Wih@x computed by batched 512-col matmuls into a PSUM
bank (start=True); recurrence matmul Whh@h_{t-1} accumulates into the
bank slice (start=False); ScalarE Tanh(psum + b) writes h_t back to the
activation buffer (in place).
"""

import ml_dtypes
import numpy as np

BF16 = ml_dtypes.bfloat16

T, B, H, EMB, OUT = 1024, 256, 128, 10, 8
DIL = (1, 2, 4, 8, 16)
NCORES = 8
BL = B // NCORES           # 32 batch per core
COLS = T * BL              # 32768 columns
NSTRIP = 4                 # x0 packed as 4 strips of 32 partitions
STRIP_COLS = COLS // NSTRIP  # 8192
BANK = 512                 # fp32 cols per PSUM bank
NCHUNK = COLS // BANK      # 64 chunks per layer
PROJ_COLS = 10 * BL        # last 10 timesteps

_cache = {}


def _build():
    import concourse.mybir as mybir
    import concourse.tile as tile
    from concourse import bacc

    f32 = mybir.dt.float32
    MMDT = mybir.dt.bfloat16
    AF = mybir.ActivationFunctionType

    from contextlib import ExitStack

    nc = bacc.Bacc(None, target_bir_lowering=False, debug=False)
    with tile.TileContext(nc) as tc, ExitStack() as es:
        if True:
            dram = es.enter_context(tc.tile_pool(name="dram", bufs=1, space="DRAM"))
            x0_d = dram.tile([128, STRIP_COLS], MMDT, kind="ExternalInput", uniquify=False, name="x0")
            w0_d = dram.tile([128, H], MMDT, kind="ExternalInput", uniquify=False, name="w0T")
            wih_d = dram.tile([128, 4 * H], MMDT, kind="ExternalInput", uniquify=False, name="wihT")
            whh_d = dram.tile([128, 5 * H], MMDT, kind="ExternalInput", uniquify=False, name="whhT")
            bs_d = dram.tile([128, 5], f32, kind="ExternalInput", uniquify=False, name="bsum")
            wp_d = dram.tile([128, OUT], MMDT, kind="ExternalInput", uniquify=False, name="wpT")
            bp_d = dram.tile([OUT, 1], f32, kind="ExternalInput", uniquify=False, name="bp")
            y_d = dram.tile([OUT, PROJ_COLS], f32, kind="ExternalOutput", uniquify=False, name="y")

            cpool = es.enter_context(tc.tile_pool(name="const", bufs=1))
            x0 = cpool.tile([128, STRIP_COLS], MMDT, name="x0sb")
            w0 = cpool.tile([128, H], MMDT, name="w0sb")
            wih = cpool.tile([128, 4 * H], MMDT, name="wihsb")
            whh = cpool.tile([128, 5 * H], MMDT, name="whhsb")
            bs = cpool.tile([128, 5], f32, name="bssb")
            wp = cpool.tile([128, OUT], MMDT, name="wpsb")
            bp = cpool.tile([OUT, 1], f32, name="bpsb")
            A = cpool.tile([128, COLS], MMDT, name="acts")
            ysb = cpool.tile([OUT, PROJ_COLS], f32, name="ysb")

            # weight/bias loads first, then x0 strip loads (layer 0 consumes
            # strip s only after strip s-1's chunks, so split the DMA)
            nc.sync.dma_start(w0[:], w0_d[:])
            nc.sync.dma_start(wih[:], wih_d[:])
            nc.sync.dma_start(whh[:], whh_d[:])
            nc.sync.dma_start(bs[:], bs_d[:])
            nc.sync.dma_start(wp[:], wp_d[:])
            nc.sync.dma_start(bp[:], bp_d[:])
            for s in range(NSTRIP):
                q = STRIP_COLS // NSTRIP
                for ss in range(NSTRIP):
                    nc.sync.dma_start(
                        x0[32 * s : 32 * s + EMB, ss * q : (ss + 1) * q],
                        x0_d[32 * s : 32 * s + EMB, ss * q : (ss + 1) * q],
                    )

            pools = []
            for l, nb in enumerate((2, 2, 2, 1, 1)):
                pools.append(
                    es.enter_context(
                        tc.tile_pool(name=f"ps{l}", bufs=nb, space="PSUM")
                    )
                )

            for l in range(5):
                d = DIL[l]
                R = d * BL                # cols per step
                steps = T // d
                spc = BANK // R           # steps per chunk (>=1)
                whh_l = whh[:, l * H : (l + 1) * H]
                bias_l = bs[:, l : l + 1]
                for c in range(NCHUNK):
                    pt = pools[l].tile([128, BANK], f32, name=f"psum{l}", tag=f"pt{l}")
                    lo = c * BANK
                    t0 = c * spc
                    nrec = spc - 1 if t0 == 0 else spc
                    # pre-activation matmul(s) for this bank
                    if l == 0:
                        s = lo // STRIP_COLS
                        off = lo % STRIP_COLS
                        nc.tensor.matmul(
                            pt[:],
                            w0[32 * s : 32 * s + EMB, :],
                            x0[32 * s : 32 * s + EMB, off : off + BANK],
                            start=True,
                            stop=(nrec == 0),
                            tile_position=(32 * s, 0),
                        )
                    else:
                        nc.tensor.matmul(
                            pt[:],
                            wih[:, (l - 1) * H : l * H],
                            A[:, lo : lo + BANK],
                            start=True,
                            stop=(nrec == 0),
                        )
                    for k in range(spc):
                        t = t0 + k
                        sl = pt[:, k * R : (k + 1) * R]
                        if t > 0:
                            nc.tensor.matmul(
                                sl,
                                whh_l,
                                A[:, (t - 1) * R : t * R],
                                start=False,
                                stop=(k == spc - 1),
                            )
                        nc.scalar.activation(
                            A[:, t * R : (t + 1) * R], sl, AF.Tanh, bias=bias_l
                        )

            # projection: y = Wp @ acts[:, -10 steps] + bp
            pp = pools[0].tile([OUT, BANK], f32, name="psproj", tag="pt0")
            nc.tensor.matmul(
                pp[:, :PROJ_COLS],
                wp[:],
                A[:, COLS - PROJ_COLS :],
                start=True,
                stop=True,
            )
            nc.scalar.activation(ysb[:], pp[:, :PROJ_COLS], AF.Identity, bias=bp[:])
            nc.sync.dma_start(y_d[:], ysb[:])

    nc.compile()
    return nc


def _get_nc():
    if "nc" not in _cache:
        _cache["nc"] = _build()
    return _cache["nc"]


def _prep_inputs(input, embed, Wih0, Wih, Whh, bih, bhh, Wp, bp):
    input = np.asarray(input)
    embed = np.asarray(embed, np.float32)
    b = (np.asarray(bih, np.float32) + np.asarray(bhh, np.float32))  # [5, H]

    w0T = np.zeros((128, H), np.float32)
    for s in range(NSTRIP):
        w0T[32 * s : 32 * s + EMB, :] = np.asarray(Wih0, np.float32).T
    wihT = np.concatenate(
        [np.asarray(Wih[i], np.float32).T for i in range(4)], axis=1
    )  # [128, 4H]
    whhT = np.concatenate(
        [np.asarray(Whh[i], np.float32).T for i in range(5)], axis=1
    )  # [128, 5H]
    bsum = np.ascontiguousarray(b.T)  # [H, 5] -> [128, 5]
    wpT = np.ascontiguousarray(np.asarray(Wp, np.float32).T)  # [128, 8]
    bpc = np.asarray(bp, np.float32).reshape(OUT, 1)

    shared = dict(
        w0T=w0T.astype(BF16),
        wihT=np.ascontiguousarray(wihT).astype(BF16),
        whhT=np.ascontiguousarray(whhT).astype(BF16),
        bsum=bsum, wpT=wpT.astype(BF16), bp=bpc,
    )

    in_maps = []
    for core in range(NCORES):
        tok = input[:, core * BL : (core + 1) * BL]          # [T, BL]
        xe = embed[tok]                                      # [T, BL, EMB]
        xe = xe.transpose(2, 0, 1).reshape(EMB, COLS)        # col = tau*BL + b
        x0 = np.zeros((128, STRIP_COLS), BF16)
        for s in range(NSTRIP):
            x0[32 * s : 32 * s + EMB, :] = xe[:, s * STRIP_COLS : (s + 1) * STRIP_COLS]
        in_maps.append(dict(shared, x0=x0))
    return in_maps


def kernel(input, embed, Wih0, Wih, Whh, bih, bhh, Wp, bp):
    from concourse.bass_utils import run_bass_kernel_spmd

    nc = _get_nc()
    in_maps = _prep_inputs(input, embed, Wih0, Wih, Whh, bih, bhh, Wp, bp)
    res = run_bass_kernel_spmd(nc, in_maps, core_ids=list(range(NCORES)))
    _cache["last_res"] = res
    out = np.empty((10, B, OUT), np.float32)
    for core in range(NCORES):
        y = res.results[core]["y"]                 # [8, 10*BL]
        out[:, core * BL : (core + 1) * BL, :] = (
            y.reshape(OUT, 10, BL).transpose(1, 2, 0)
        )
    return out


# revision 12
# speedup vs baseline: 1.9688x; 1.0006x over previous
"""Dilated tanh-RNN stack (5 layers, dil 1,2,4,8,16) on 8 trn2 cores.

Sharding: data-parallel over batch B=256 -> 32 per core. Time recurrence
is local. Layout on device: feature-major [H=128 partitions, T*BL cols],
col = tau*BL + b  (tau = original time). With this layout the dilation
reshape [T,B,C]->[T/d, d*B, C] is the identity on columns, so all five
layers operate in place on one SBUF buffer.

Per layer: pre = Wih@x computed by batched 512-col matmuls into a PSUM
bank (start=True); recurrence matmul Whh@h_{t-1} accumulates into the
bank slice (start=False); ScalarE Tanh(psum + b) writes h_t back to the
activation buffer (in place).
"""

import ml_dtypes
import numpy as np

BF16 = ml_dtypes.bfloat16

T, B, H, EMB, OUT = 1024, 256, 128, 10, 8
DIL = (1, 2, 4, 8, 16)
NCORES = 8
BL = B // NCORES           # 32 batch per core
COLS = T * BL              # 32768 columns
NSTRIP = 4                 # x0 packed as 4 strips of 32 partitions
STRIP_COLS = COLS // NSTRIP  # 8192
BANK = 512                 # fp32 cols per PSUM bank
NCHUNK = COLS // BANK      # 64 chunks per layer
PROJ_COLS = 10 * BL        # last 10 timesteps

_cache = {}


def _build():
    import concourse.mybir as mybir
    import concourse.tile as tile
    from concourse import bacc

    f32 = mybir.dt.float32
    MMDT = mybir.dt.bfloat16
    AF = mybir.ActivationFunctionType

    from contextlib import ExitStack

    nc = bacc.Bacc(None, target_bir_lowering=False, debug=False)
    with tile.TileContext(nc) as tc, ExitStack() as es:
        if True:
            dram = es.enter_context(tc.tile_pool(name="dram", bufs=1, space="DRAM"))
            x0_d = dram.tile([128, STRIP_COLS], MMDT, kind="ExternalInput", uniquify=False, name="x0")
            w0_d = dram.tile([128, H], MMDT, kind="ExternalInput", uniquify=False, name="w0T")
            wih_d = dram.tile([128, 4 * H], MMDT, kind="ExternalInput", uniquify=False, name="wihT")
            whh_d = dram.tile([128, 5 * H], MMDT, kind="ExternalInput", uniquify=False, name="whhT")
            bs_d = dram.tile([128, 5], f32, kind="ExternalInput", uniquify=False, name="bsum")
            wp_d = dram.tile([128, OUT], MMDT, kind="ExternalInput", uniquify=False, name="wpT")
            bp_d = dram.tile([OUT, 1], f32, kind="ExternalInput", uniquify=False, name="bp")
            y_d = dram.tile([OUT, PROJ_COLS], f32, kind="ExternalOutput", uniquify=False, name="y")

            cpool = es.enter_context(tc.tile_pool(name="const", bufs=1))
            x0 = cpool.tile([128, STRIP_COLS], MMDT, name="x0sb")
            w0 = cpool.tile([128, H], MMDT, name="w0sb")
            wih = cpool.tile([128, 4 * H], MMDT, name="wihsb")
            whh = cpool.tile([128, 5 * H], MMDT, name="whhsb")
            bs = cpool.tile([128, 5], f32, name="bssb")
            wp = cpool.tile([128, OUT], MMDT, name="wpsb")
            bp = cpool.tile([OUT, 1], f32, name="bpsb")
            A = cpool.tile([128, COLS], MMDT, name="acts")
            ysb = cpool.tile([OUT, PROJ_COLS], f32, name="ysb")

            # x0 strips on the gpsimd DMA queue (first piece gates layer 0
            # chunk 0); weights/biases on sync, critical-path ones first
            for s in range(NSTRIP):
                q = STRIP_COLS // NSTRIP
                for ss in range(NSTRIP):
                    nc.gpsimd.dma_start(
                        x0[32 * s : 32 * s + EMB, ss * q : (ss + 1) * q],
                        x0_d[32 * s : 32 * s + EMB, ss * q : (ss + 1) * q],
                    )
            nc.sync.dma_start(w0[:], w0_d[:])
            nc.sync.dma_start(whh[:], whh_d[:])
            nc.sync.dma_start(bs[:], bs_d[:])
            nc.sync.dma_start(wih[:], wih_d[:])
            nc.sync.dma_start(wp[:], wp_d[:])
            nc.sync.dma_start(bp[:], bp_d[:])

            pools = []
            for l, nb in enumerate((2, 2, 2, 1, 1)):
                pools.append(
                    es.enter_context(
                        tc.tile_pool(name=f"ps{l}", bufs=nb, space="PSUM")
                    )
                )

            for l in range(5):
                d = DIL[l]
                R = d * BL                # cols per step
                steps = T // d
                spc = BANK // R           # steps per chunk (>=1)
                whh_l = whh[:, l * H : (l + 1) * H]
                bias_l = bs[:, l : l + 1]
                for c in range(NCHUNK):
                    pt = pools[l].tile([128, BANK], f32, name=f"psum{l}", tag=f"pt{l}")
                    lo = c * BANK
                    t0 = c * spc
                    nrec = spc - 1 if t0 == 0 else spc
                    # pre-activation matmul(s) for this bank
                    if l == 0:
                        s = lo // STRIP_COLS
                        off = lo % STRIP_COLS
                        nc.tensor.matmul(
                            pt[:],
                            w0[32 * s : 32 * s + EMB, :],
                            x0[32 * s : 32 * s + EMB, off : off + BANK],
                            start=True,
                            stop=(nrec == 0),
                            tile_position=(32 * s, 0),
                        )
                    else:
                        nc.tensor.matmul(
                            pt[:],
                            wih[:, (l - 1) * H : l * H],
                            A[:, lo : lo + BANK],
                            start=True,
                            stop=(nrec == 0),
                        )
                    for k in range(spc):
                        t = t0 + k
                        sl = pt[:, k * R : (k + 1) * R]
                        if t > 0:
                            nc.tensor.matmul(
                                sl,
                                whh_l,
                                A[:, (t - 1) * R : t * R],
                                start=False,
                                stop=(k == spc - 1),
                            )
                        nc.scalar.activation(
                            A[:, t * R : (t + 1) * R], sl, AF.Tanh, bias=bias_l
                        )

            # projection: y = Wp @ acts[:, -10 steps] + bp
            pp = pools[0].tile([OUT, BANK], f32, name="psproj", tag="pt0")
            nc.tensor.matmul(
                pp[:, :PROJ_COLS],
                wp[:],
                A[:, COLS - PROJ_COLS :],
                start=True,
                stop=True,
            )
            nc.scalar.activation(ysb[:], pp[:, :PROJ_COLS], AF.Identity, bias=bp[:])
            nc.sync.dma_start(y_d[:], ysb[:])

    nc.compile()
    return nc


def _get_nc():
    if "nc" not in _cache:
        _cache["nc"] = _build()
    return _cache["nc"]


def _prep_inputs(input, embed, Wih0, Wih, Whh, bih, bhh, Wp, bp):
    input = np.asarray(input)
    embed = np.asarray(embed, np.float32)
    b = (np.asarray(bih, np.float32) + np.asarray(bhh, np.float32))  # [5, H]

    w0T = np.zeros((128, H), np.float32)
    for s in range(NSTRIP):
        w0T[32 * s : 32 * s + EMB, :] = np.asarray(Wih0, np.float32).T
    wihT = np.concatenate(
        [np.asarray(Wih[i], np.float32).T for i in range(4)], axis=1
    )  # [128, 4H]
    whhT = np.concatenate(
        [np.asarray(Whh[i], np.float32).T for i in range(5)], axis=1
    )  # [128, 5H]
    bsum = np.ascontiguousarray(b.T)  # [H, 5] -> [128, 5]
    wpT = np.ascontiguousarray(np.asarray(Wp, np.float32).T)  # [128, 8]
    bpc = np.asarray(bp, np.float32).reshape(OUT, 1)

    shared = dict(
        w0T=w0T.astype(BF16),
        wihT=np.ascontiguousarray(wihT).astype(BF16),
        whhT=np.ascontiguousarray(whhT).astype(BF16),
        bsum=bsum, wpT=wpT.astype(BF16), bp=bpc,
    )

    in_maps = []
    for core in range(NCORES):
        tok = input[:, core * BL : (core + 1) * BL]          # [T, BL]
        xe = embed[tok]                                      # [T, BL, EMB]
        xe = xe.transpose(2, 0, 1).reshape(EMB, COLS)        # col = tau*BL + b
        x0 = np.zeros((128, STRIP_COLS), BF16)
        for s in range(NSTRIP):
            x0[32 * s : 32 * s + EMB, :] = xe[:, s * STRIP_COLS : (s + 1) * STRIP_COLS]
        in_maps.append(dict(shared, x0=x0))
    return in_maps


def kernel(input, embed, Wih0, Wih, Whh, bih, bhh, Wp, bp):
    from concourse.bass_utils import run_bass_kernel_spmd

    nc = _get_nc()
    in_maps = _prep_inputs(input, embed, Wih0, Wih, Whh, bih, bhh, Wp, bp)
    res = run_bass_kernel_spmd(nc, in_maps, core_ids=list(range(NCORES)))
    _cache["last_res"] = res
    out = np.empty((10, B, OUT), np.float32)
    for core in range(NCORES):
        y = res.results[core]["y"]                 # [8, 10*BL]
        out[:, core * BL : (core + 1) * BL, :] = (
            y.reshape(OUT, 10, BL).transpose(1, 2, 0)
        )
    return out


# revision 14
# speedup vs baseline: 1.9711x; 1.0012x over previous
"""Dilated tanh-RNN stack (5 layers, dil 1,2,4,8,16) on 8 trn2 cores.

Sharding: data-parallel over batch B=256 -> 32 per core. Time recurrence
is local. Layout on device: feature-major [H=128 partitions, T*BL cols],
col = tau*BL + b  (tau = original time). With this layout the dilation
reshape [T,B,C]->[T/d, d*B, C] is the identity on columns, so all five
layers operate in place on one SBUF buffer.

Per layer: pre = Wih@x computed by batched 512-col matmuls into a PSUM
bank (start=True); recurrence matmul Whh@h_{t-1} accumulates into the
bank slice (start=False); ScalarE Tanh(psum + b) writes h_t back to the
activation buffer (in place).
"""

import ml_dtypes
import numpy as np

BF16 = ml_dtypes.bfloat16

T, B, H, EMB, OUT = 1024, 256, 128, 10, 8
DIL = (1, 2, 4, 8, 16)
NCORES = 8
BL = B // NCORES           # 32 batch per core
COLS = T * BL              # 32768 columns
NSTRIP = 4                 # x0 packed as 4 strips of 32 partitions
STRIP_COLS = COLS // NSTRIP  # 8192
BANK = 512                 # fp32 cols per PSUM bank
NCHUNK = COLS // BANK      # 64 chunks per layer
PROJ_COLS = 10 * BL        # last 10 timesteps

_cache = {}


def _build():
    import concourse.mybir as mybir
    import concourse.tile as tile
    from concourse import bacc

    f32 = mybir.dt.float32
    MMDT = mybir.dt.bfloat16
    AF = mybir.ActivationFunctionType

    from contextlib import ExitStack

    nc = bacc.Bacc(None, target_bir_lowering=False, debug=False)
    with tile.TileContext(nc) as tc, ExitStack() as es:
        if True:
            dram = es.enter_context(tc.tile_pool(name="dram", bufs=1, space="DRAM"))
            x0_d = dram.tile([128, STRIP_COLS], MMDT, kind="ExternalInput", uniquify=False, name="x0")
            w0_d = dram.tile([128, H], MMDT, kind="ExternalInput", uniquify=False, name="w0T")
            wih_d = dram.tile([128, 4 * H], MMDT, kind="ExternalInput", uniquify=False, name="wihT")
            whh_d = dram.tile([128, 5 * H], MMDT, kind="ExternalInput", uniquify=False, name="whhT")
            bs_d = dram.tile([128, 5], f32, kind="ExternalInput", uniquify=False, name="bsum")
            wp_d = dram.tile([128, OUT], MMDT, kind="ExternalInput", uniquify=False, name="wpT")
            bp_d = dram.tile([OUT, 1], f32, kind="ExternalInput", uniquify=False, name="bp")
            y_d = dram.tile([OUT, PROJ_COLS], f32, kind="ExternalOutput", uniquify=False, name="y")

            cpool = es.enter_context(tc.tile_pool(name="const", bufs=1))
            x0 = cpool.tile([128, STRIP_COLS], MMDT, name="x0sb")
            w0 = cpool.tile([128, H], MMDT, name="w0sb")
            wih = cpool.tile([128, 4 * H], MMDT, name="wihsb")
            whh = cpool.tile([128, 5 * H], MMDT, name="whhsb")
            bs = cpool.tile([128, 5], f32, name="bssb")
            wp = cpool.tile([128, OUT], MMDT, name="wpsb")
            bp = cpool.tile([OUT, 1], f32, name="bpsb")
            A = cpool.tile([128, COLS], MMDT, name="acts")
            ysb = cpool.tile([OUT, PROJ_COLS], f32, name="ysb")

            # x0 strips on the gpsimd DMA queue (first piece gates layer 0
            # chunk 0); weights/biases on sync, critical-path ones first
            for s in range(NSTRIP):
                q = STRIP_COLS // NSTRIP
                for ss in range(NSTRIP):
                    nc.gpsimd.dma_start(
                        x0[32 * s : 32 * s + EMB, ss * q : (ss + 1) * q],
                        x0_d[32 * s : 32 * s + EMB, ss * q : (ss + 1) * q],
                    )
            nc.sync.dma_start(w0[:], w0_d[:])
            nc.sync.dma_start(whh[:], whh_d[:])
            nc.sync.dma_start(bs[:], bs_d[:])
            nc.sync.dma_start(wih[:], wih_d[:])
            nc.sync.dma_start(wp[:], wp_d[:])
            nc.sync.dma_start(bp[:], bp_d[:])

            pools = []
            for l, nb in enumerate((2, 2, 2, 1, 1)):
                pools.append(
                    es.enter_context(
                        tc.tile_pool(name=f"ps{l}", bufs=nb, space="PSUM")
                    )
                )

            for l in range(5):
                d = DIL[l]
                R = d * BL                # cols per step
                steps = T // d
                spc = BANK // R           # steps per chunk (>=1)
                whh_l = whh[:, l * H : (l + 1) * H]
                bias_l = bs[:, l : l + 1]
                for c in range(NCHUNK):
                    pt = pools[l].tile([128, BANK], f32, name=f"psum{l}", tag=f"pt{l}")
                    lo = c * BANK
                    t0 = c * spc
                    nrec = spc - 1 if t0 == 0 else spc
                    # pre-activation matmul(s) for this bank
                    if l == 0:
                        s = lo // STRIP_COLS
                        off = lo % STRIP_COLS
                        nc.tensor.matmul(
                            pt[:],
                            w0[32 * s : 32 * s + EMB, :],
                            x0[32 * s : 32 * s + EMB, off : off + BANK],
                            start=True,
                            stop=(nrec == 0),
                            tile_position=(32 * s, 0),
                        )
                    else:
                        nc.tensor.matmul(
                            pt[:],
                            wih[:, (l - 1) * H : l * H],
                            A[:, lo : lo + BANK],
                            start=True,
                            stop=(nrec == 0),
                        )
                    for k in range(spc):
                        t = t0 + k
                        sl = pt[:, k * R : (k + 1) * R]
                        if t > 0:
                            nc.tensor.matmul(
                                sl,
                                whh_l,
                                A[:, (t - 1) * R : t * R],
                                start=False,
                                stop=(k == spc - 1),
                            )
                        nc.scalar.activation(
                            A[:, t * R : (t + 1) * R], sl, AF.Tanh, bias=bias_l
                        )

            # projection: y = Wp @ acts[:, -10 steps] + bp
            pp = pools[0].tile([OUT, BANK], f32, name="psproj", tag="pt0")
            nc.tensor.matmul(
                pp[:, :PROJ_COLS],
                wp[:],
                A[:, COLS - PROJ_COLS :],
                start=True,
                stop=True,
            )
            nc.scalar.activation(ysb[:], pp[:, :PROJ_COLS], AF.Identity, bias=bp[:])
            nc.sync.dma_start(y_d[:], ysb[:])

    nc.compile()
    return nc


def _get_nc():
    if "nc" not in _cache:
        _cache["nc"] = _build()
    return _cache["nc"]


def _prep_inputs(input, embed, Wih0, Wih, Whh, bih, bhh, Wp, bp):
    input = np.asarray(input)
    embed = np.asarray(embed, np.float32)
    b = (np.asarray(bih, np.float32) + np.asarray(bhh, np.float32))  # [5, H]

    w0T = np.zeros((128, H), np.float32)
    for s in range(NSTRIP):
        w0T[32 * s : 32 * s + EMB, :] = np.asarray(Wih0, np.float32).T
    wihT = np.concatenate(
        [np.asarray(Wih[i], np.float32).T for i in range(4)], axis=1
    )  # [128, 4H]
    whhT = np.concatenate(
        [np.asarray(Whh[i], np.float32).T for i in range(5)], axis=1
    )  # [128, 5H]
    bsum = np.ascontiguousarray(b.T)  # [H, 5] -> [128, 5]
    wpT = np.ascontiguousarray(np.asarray(Wp, np.float32).T)  # [128, 8]
    bpc = np.asarray(bp, np.float32).reshape(OUT, 1)

    shared = dict(
        w0T=w0T.astype(BF16),
        wihT=np.ascontiguousarray(wihT).astype(BF16),
        whhT=np.ascontiguousarray(whhT).astype(BF16),
        bsum=bsum, wpT=wpT.astype(BF16), bp=bpc,
    )

    in_maps = []
    for core in range(NCORES):
        tok = input[:, core * BL : (core + 1) * BL]          # [T, BL]
        xe = embed[tok]                                      # [T, BL, EMB]
        xe = xe.transpose(2, 0, 1).reshape(EMB, COLS)        # col = tau*BL + b
        x0 = np.zeros((128, STRIP_COLS), BF16)
        for s in range(NSTRIP):
            x0[32 * s : 32 * s + EMB, :] = xe[:, s * STRIP_COLS : (s + 1) * STRIP_COLS]
        in_maps.append(dict(shared, x0=x0))
    return in_maps


def kernel(input, embed, Wih0, Wih, Whh, bih, bhh, Wp, bp):
    from concourse.bass_utils import run_bass_kernel_spmd

    nc = _get_nc()
    in_maps = _prep_inputs(input, embed, Wih0, Wih, Whh, bih, bhh, Wp, bp)
    res = run_bass_kernel_spmd(nc, in_maps, core_ids=list(range(NCORES)))
    _cache["last_res"] = res
    out = np.empty((10, B, OUT), np.float32)
    for core in range(NCORES):
        y = res.results[core]["y"]                 # [8, 10*BL]
        out[:, core * BL : (core + 1) * BL, :] = (
            y.reshape(OUT, 10, BL).transpose(1, 2, 0)
        )
    return out
